# revision 29
# baseline (speedup 1.0000x reference)
"""DETR-style matcher cost matrix on 8 Trainium2 NeuronCores.

cost[b, g, p] = -pred_cls[b, p, g]
                + mean(|pred_box[p] - gt_box[g]|)          (L1, 4 coords)
                + 1 - IoU + (area_c - union)/(area_c+eps)  (GIoU loss)
masked to zero where gt_validity[b, g] == 0.

Sharding: data-parallel over batch, 4 batches per core (B=32, 8 cores).

Layout per (batch, gt-tile of 128): [128 part = gt rows, 900 free = preds].
Per-pred values enter as partition-broadcast maps (fp16 for 2x DVE modes),
per-gt values as [128,1] fp32 scalars.  Identities used:
  wi0   = min(Px2,Gx2) - max(Px1,Gx1)        pre-relu intersection width
  wc    = (wp + wg) - wi0                    enclosing-box width
  l1*4  = (wc + hc) - (wi0 + hi0) = (wp+wg+hp+hg) - 2*(wi0+hi0)
  inter = relu(wi0)*relu(hi0)
  union = area_p + area_g - inter
  t2    = (area_c - union)/(area_c) ~= 1 - union/area_c   (eps folded)
  cost  = V*(0.25*SWH - 0.5*s2 + 2 - iou - union/area_c) - V*clsT
The fp32 division tail uses RECIPROCAL_APPROX_FAST (~51 ULP).
pred_cls.T comes via PE transposes into PSUM; ScalarE folds it to
V*(2 - clsT) in SBUF so the final combine is one scalar_tensor_tensor.
"""

import numpy as np

B, Q = 32, 900
N_CORES = 8
B_PER = B // N_CORES
EPS = 1e-7
GT = 8  # gt tiles per batch: 7 full x128 + 1 of 4 rows
PT = 8  # pred chunks of 128 (last = 4)

USE_CUSTOM = True  # authored fused DVE ops (W0_IOU_ANT / RELUMUL_ANT)
USE_GP = False  # offload some fp32 tail ops to GpSimd

_cached = {}


def _split_multi_waits(nc):
    """This neuronxcc build rejects >1 sync-wait per instruction. Split any
    instruction carrying N>1 waits by inserting N-1 wait-carrier nops before
    it on the same (in-order) engine stream."""
    import concourse.mybir as mybir

    for fn in nc.m.functions:
        for bb in fn.blocks:
            out = []
            for ins in bb.instructions:
                si = getattr(ins, "sync_info", None)
                waits = list(si.on_wait) if (si and si.on_wait) else []
                if len(waits) > 1:
                    si.on_wait = [waits[-1]]
                    for j, w in enumerate(waits[:-1]):
                        nop = mybir.InstNoOp(name=f"{ins.name}-sw{j}", ins=[], outs=[])
                        nop.engine = ins.engine
                        nop.sync_info = mybir.SyncInfo(on_wait=[w], on_update=[])
                        out.append(nop)
                out.append(ins)
            bb.instructions[:] = out


def _ensure_custom_ops():
    """Author two fused DVE ops and register them in dve_ops' tables:
      W0_IOU_ANT:  out = min(in0, s0) - max(in1, s1)
      RELUMUL_ANT: out = relu(in0) * relu(in1)
    """
    from concourse import dve_ops
    from concourse.dve_spec import Spec, Src0, Src1, C0, C1, minn, maxx, relu
    from concourse.dve_spec import lower, _has_src1
    from concourse.dve_uop import DveOpSpec

    if "W0_IOU_ANT" in dve_ops._SUB_OPCODE_FOR_NAME:
        return

    from concourse.dve_spec import C2

    def author(name, body, ref):
        spec = Spec(body=body, reference=ref)
        row = max(dve_ops._SUB_OPCODE_FOR_NAME.values()) + 1
        shas = {}
        for ver in ("v3", "v4"):
            uops = lower(spec, ver=ver)
            s = DveOpSpec(name=name, opcode=row, uops=uops, rd1_en=_has_src1(spec))
            shas[ver] = s.sha(ver)
        op = dve_ops.DveOp(name, spec, False, shas)
        dve_ops.OPS.append(op)
        dve_ops.CUSTOM_DVE_SPECS[name] = spec
        dve_ops._SUB_OPCODE_FOR_NAME[name] = row
        return op

    w0 = author(
        "W0_IOU_ANT",
        (minn(Src0, C0) - maxx(Src1, C1)) * C2,
        lambda in0, in1, s0, s1, imm2: (np.minimum(in0, s0) - np.maximum(in1, s1))
        * imm2,
    )
    rm = author(
        "RELUMUL_ANT",
        relu(Src0) * relu(Src1) * C2,
        lambda in0, in1, s0, s1, imm2: np.maximum(in0, 0.0)
        * np.maximum(in1, 0.0)
        * imm2,
    )
    return w0, rm


def _by_name(dve_ops, name):
    for op in dve_ops.OPS:
        if op.name == name:
            return op
    raise KeyError(name)


def _build_nc():
    import concourse.bass as bass
    from concourse import mybir, dve_ops
    from concourse.tile import TileContext
    from concourse.masks import make_identity

    if USE_CUSTOM:
        _ensure_custom_ops()
        W0 = _by_name(dve_ops, "W0_IOU_ANT")
        RM = _by_name(dve_ops, "RELUMUL_ANT")

    f32 = mybir.dt.float32
    f16 = mybir.dt.float16
    Alu = mybir.AluOpType
    Act = mybir.ActivationFunctionType

    nc = bass.Bass()
    pb_d = nc.dram_tensor("pred_boxes", [B_PER, Q, 4], f32, kind="ExternalInput")
    gb_d = nc.dram_tensor("gt_boxes", [B_PER, Q, 4], f32, kind="ExternalInput")
    cls_d = nc.dram_tensor("pred_cls", [B_PER, Q, Q], f32, kind="ExternalInput")
    val_d = nc.dram_tensor("validity", [B_PER, Q], f32, kind="ExternalInput")
    cost_d = nc.dram_tensor("cost", [B_PER, Q, Q], f32, kind="ExternalOutput")

    with TileContext(nc) as tc:
        with (
            tc.tile_pool(name="const", bufs=1) as constp,
            tc.tile_pool(name="batch", bufs=2) as batchp,
            tc.tile_pool(name="cls", bufs=3) as clsp,
            tc.tile_pool(name="chain", bufs=2) as chp,
            tc.tile_pool(name="outp", bufs=3) as outp,
            tc.tile_pool(name="psum", bufs=2, space="PSUM") as psp,
        ):
            ident = constp.tile([128, 128], f32)
            make_identity(nc, ident)
            onesrow = constp.tile([1, Q], f32)
            nc.gpsimd.memset(onesrow[:], 1.0)
            neg2 = constp.tile([1, 1], f32)
            nc.gpsimd.memset(neg2[:], -2.0)

            gp = nc.gpsimd if USE_GP else nc.vector

            # widths are carried scaled by SC=256 in fp16 to stay clear of
            # fp16 subnormals; SC folds back out via imm scalars downstream.
            SC = 256.0 if USE_CUSTOM else 1.0
            ISC2 = 1.0 / (SC * SC)
            hdt = f16 if USE_CUSTOM else f32

            def emit_chain(m4c, WPhX, HPhX, SPh4X, APmX, S, clsV, mode="psacc"):
                """One [128 gt x 900 pred] unit chain; returns the out tile."""
                stt = nc.vector.scalar_tensor_tensor
                wi0 = chp.tile([128, Q], hdt, tag="wi0")
                hi0 = chp.tile([128, Q], hdt, tag="hi0")
                if USE_CUSTOM:
                    nc.vector._custom_dve(
                        W0, out=wi0[:], in0=m4c[2], in1=m4c[0],
                        s0=S["Gx2"], s1=S["Gx1"], imm2=SC,
                    )
                    nc.vector._custom_dve(
                        W0, out=hi0[:], in0=m4c[3], in1=m4c[1],
                        s0=S["Gy2"], s1=S["Gy1"], imm2=SC,
                    )
                else:
                    Mx1 = chp.tile([128, Q], f32, tag="Mx1")
                    nc.vector.tensor_scalar_max(Mx1[:], m4c[0], S["Gx1"])
                    mx2 = chp.tile([128, Q], f32, tag="mx2")
                    nc.vector.tensor_scalar_min(mx2[:], m4c[2], S["Gx2"])
                    nc.vector.tensor_sub(wi0[:], mx2[:], Mx1[:])
                    My1 = chp.tile([128, Q], f32, tag="My1")
                    nc.vector.tensor_scalar_max(My1[:], m4c[1], S["Gy1"])
                    my2 = chp.tile([128, Q], f32, tag="my2")
                    nc.vector.tensor_scalar_min(my2[:], m4c[3], S["Gy2"])
                    nc.vector.tensor_sub(hi0[:], my2[:], My1[:])

                # s2 = wi0 + hi0 on the DMA engines (CCE inline add)
                s2 = chp.tile([128, Q], hdt, tag="s2")
                nc.gpsimd.dma_start(out=s2[:], in_=wi0[:])
                nc.gpsimd.dma_start(out=s2[:], in_=hi0[:], accum_op=Alu.add)

                W = chp.tile([128, Q], hdt, tag="W")
                nc.scalar.activation(W[:], WPhX[:], Act.Identity, bias=S["WGs"])
                wc = chp.tile([128, Q], hdt, tag="wc")
                nc.vector.tensor_sub(wc[:], W[:], wi0[:])
                H = chp.tile([128, Q], hdt, tag="H")
                nc.scalar.activation(H[:], HPhX[:], Act.Identity, bias=S["HGs"])
                hc = chp.tile([128, Q], hdt, tag="hc")
                nc.vector.tensor_sub(hc[:], H[:], hi0[:])

                inter = chp.tile([128, Q], f32, tag="inter")
                areac = chp.tile([128, Q], f32, tag="areac")
                if USE_CUSTOM:
                    # whole division cluster SC^2-scaled; ratios cancel
                    nc.vector._custom_dve(
                        RM, out=inter[:], in0=wi0[:], in1=hi0[:], imm2=1.0
                    )
                    nc.vector.tensor_mul(areac[:], wc[:], hc[:])
                elif False:
                    wiR = chp.tile([128, Q], f32, tag="wiR")
                    nc.vector.tensor_scalar_max(wiR[:], wi0[:], 0.0)
                    hiR = chp.tile([128, Q], f32, tag="hiR")
                    nc.vector.tensor_scalar_max(hiR[:], hi0[:], 0.0)
                    nc.vector.tensor_mul(inter[:], wiR[:], hiR[:])
                    nc.vector.tensor_mul(areac[:], wc[:], hc[:])
                union = chp.tile([128, Q], f32, tag="union")
                stt(union[:], APmX[:], S["AGe"], inter[:], Alu.add, Alu.subtract)

                rcu = chp.tile([128, Q], f32, tag="rcu")
                nc.scalar.activation(rcu[:], union[:], Act.Ln)
                nc.scalar.activation(rcu[:], rcu[:], Act.Exp, scale=-1.0)
                rca = chp.tile([128, Q], f32, tag="rca")
                nc.scalar.activation(rca[:], areac[:], Act.Ln)
                nc.scalar.activation(rca[:], rca[:], Act.Exp, scale=-1.0)

                # c1 = inter/union + union/areac; the add rides the DMA CCE
                c1 = chp.tile([128, Q], f32, tag="c1")
                nc.vector.tensor_mul(c1[:], inter[:], rcu[:])
                t2m = chp.tile([128, Q], f32, tag="t2m")
                nc.vector.tensor_mul(t2m[:], union[:], rca[:])
                nc.gpsimd.dma_start(out=c1[:], in_=t2m[:], accum_op=Alu.add)

                out = outp.tile([128, Q], f32, tag="out")
                if mode == "psacc":
                    # clsV = V*(SWH4 + 2 - clsT) from the PE-accumulated PSUM
                    q = chp.tile([128, Q], f32, tag="q")
                    stt(q[:], s2[:], 0.5 / SC, c1[:], Alu.mult, Alu.add)
                    stt(out[:], q[:], S["negV"], clsV[:], Alu.mult, Alu.add)
                else:
                    # clsV = V*(2 - clsT); l1 map terms still on the DVE side
                    SWH4 = constp.tile([128, Q], hdt, tag="SWH4")
                    nc.scalar.activation(
                        SWH4[:], SPh4X[:], Act.Identity, bias=S["SG4"]
                    )
                    c3 = constp.tile([128, Q], f32, tag="c3")
                    stt(c3[:], s2[:], -0.5 / SC, SWH4[:], Alu.mult, Alu.add)
                    c4 = constp.tile([128, Q], f32, tag="c4")
                    nc.vector.tensor_sub(c4[:], c3[:], c1[:])
                    stt(out[:], c4[:], S["V"], clsV[:], Alu.mult, Alu.add)
                return out

            def derive_pred_maps(m4c, tagsuf, pool):
                WPhX = pool.tile([128, Q], hdt, tag="WPh" + tagsuf)
                HPhX = pool.tile([128, Q], hdt, tag="HPh" + tagsuf)
                if USE_CUSTOM:
                    nc.vector._custom_dve(
                        W0, out=WPhX[:], in0=m4c[2], in1=m4c[0],
                        s0=1e30, s1=-1e30, imm2=SC,
                    )
                    nc.vector._custom_dve(
                        W0, out=HPhX[:], in0=m4c[3], in1=m4c[1],
                        s0=1e30, s1=-1e30, imm2=SC,
                    )
                else:
                    nc.vector.tensor_sub(WPhX[:], m4c[2], m4c[0])
                    nc.vector.tensor_sub(HPhX[:], m4c[3], m4c[1])
                SPsX = chp.tile([128, Q], hdt, tag="SPs")
                nc.vector.tensor_add(SPsX[:], WPhX[:], HPhX[:])
                SPh4X = pool.tile([128, Q], hdt, tag="SPh4" + tagsuf)
                nc.vector.tensor_scalar_mul(SPh4X[:], SPsX[:], 0.25 / SC)
                APmX = pool.tile([128, Q], f32, tag="APm" + tagsuf)
                if USE_CUSTOM:
                    nc.vector._custom_dve(
                        RM, out=APmX[:], in0=WPhX[:], in1=HPhX[:], imm2=1.0
                    )
                else:
                    nc.vector.tensor_mul(APmX[:], WPhX[:], HPhX[:])
                return WPhX, HPhX, SPh4X, APmX

            def derive_gt_scalars(gsrc, vsrc, n, tagsuf, pool):
                """gsrc [128,n,4] coords, vsrc [128,n] validity -> scalar tiles."""
                WGX = pool.tile([128, n], f32, tag="WG" + tagsuf)
                nc.vector.tensor_sub(WGX[:], gsrc[:, :, 2], gsrc[:, :, 0])
                HGX = pool.tile([128, n], f32, tag="HG" + tagsuf)
                nc.vector.tensor_sub(HGX[:], gsrc[:, :, 3], gsrc[:, :, 1])
                WGsX = pool.tile([128, n], f32, tag="WGs" + tagsuf)
                nc.vector.tensor_scalar_mul(WGsX[:], WGX[:], SC)
                HGsX = pool.tile([128, n], f32, tag="HGs" + tagsuf)
                nc.vector.tensor_scalar_mul(HGsX[:], HGX[:], SC)
                AGeX = pool.tile([128, n], f32, tag="AGe" + tagsuf)
                nc.vector.tensor_mul(AGeX[:], WGsX[:], HGsX[:])
                nc.vector.tensor_scalar_add(AGeX[:], AGeX[:], float(EPS) * SC * SC)
                SG4X = pool.tile([128, n], f32, tag="SG4" + tagsuf)
                nc.vector.tensor_add(SG4X[:], WGX[:], HGX[:])
                nc.vector.tensor_scalar_mul(SG4X[:], SG4X[:], 0.25)
                negVX = pool.tile([128, n], f32, tag="negV" + tagsuf)
                nc.vector.tensor_scalar_mul(negVX[:], vsrc[:], -1.0)
                twoVX = pool.tile([128, n], f32, tag="twoV" + tagsuf)
                nc.vector.tensor_scalar_mul(twoVX[:], vsrc[:], 2.0)
                return dict(WG=WGX, HG=HGX, AGe=AGeX, SG4=SG4X, WGs=WGsX,
                            HGs=HGsX, negV=negVX, twoV=twoVX)

            def scalars_at(D, gsrc, vsrc, t):
                return {
                    "Gx1": gsrc[:, t, 0:1], "Gy1": gsrc[:, t, 1:2],
                    "Gx2": gsrc[:, t, 2:3], "Gy2": gsrc[:, t, 3:4],
                    "WGs": D["WGs"][:, t : t + 1], "HGs": D["HGs"][:, t : t + 1],
                    "AGe": D["AGe"][:, t : t + 1], "SG4": D["SG4"][:, t : t + 1],
                    "V": vsrc[:, t : t + 1], "negV": D["negV"][:, t : t + 1],
                }

            m4_batches = []
            for b in range(B_PER):
                # ---- per-batch: pred maps (fp32 coords, partition-bcast) ----
                map4 = constp.tile([128, 4 * Q], f32, tag="map4")
                src = pb_d[b][:].flatten()  # [3600]
                bcast = bass.AP(
                    tensor=src.tensor, offset=src.offset, ap=[[0, 128]] + list(src.ap)
                )
                nc.sync.dma_start(out=map4[:], in_=bcast)
                m4 = map4[:].rearrange("p (q c) -> p c q", c=4)
                m4c = [m4[:, c, :] for c in range(4)]
                WPh, HPh, SPh4, APm = derive_pred_maps(m4c, "", batchp)

                # ---- per-batch: gt scalars ---------------------------------
                gall = batchp.tile([128, 7, 4], f32, tag="gall")
                nc.sync.dma_start(
                    out=gall[:],
                    in_=gb_d[b, 0:896, :].rearrange("(t p) c -> p t c", p=128),
                )
                vall = batchp.tile([128, 7], f32, tag="vall")
                nc.sync.dma_start(
                    out=vall[:],
                    in_=val_d[b, 0:896].rearrange("(t p) -> p t", p=128),
                )
                D = derive_gt_scalars(gall, vall, 7, "", batchp)

                # ---- 7 full gt-tile units ----------------------------------
                for t in range(7):
                    g0 = t * 128
                    clsin = clsp.tile([128, PT, 128], f32, tag="clsin")
                    for k in range(PT):
                        p0 = k * 128
                        pw = 128 if k < 7 else 4
                        nc.sync.dma_start(
                            out=clsin[0:pw, k, :],
                            in_=cls_d[b, p0 : p0 + pw, g0 : g0 + 128],
                        )
                    psA = psp.tile([128, 512], f32, tag="psA")
                    psB = psp.tile([128, 388], f32, tag="psB")
                    for k in range(PT):
                        p0 = k * 128
                        pw = 128 if k < 7 else 4
                        dst = (
                            psA[:, p0 : p0 + pw]
                            if p0 < 512
                            else psB[:, p0 - 512 : p0 - 512 + pw]
                        )
                        nc.tensor.transpose(dst, clsin[0:pw, k, :], ident[0:pw, 0:pw])

                    negVt = D["negV"][:, t : t + 1]
                    twoVt = D["twoV"][:, t : t + 1]
                    clsV = chp.tile([128, Q], f32, tag="clsV")
                    nc.scalar.activation(
                        clsV[:, 0:512], psA[:, :], Act.Identity, bias=twoVt, scale=negVt
                    )
                    nc.scalar.activation(
                        clsV[:, 512:900], psB[:, :], Act.Identity, bias=twoVt, scale=negVt
                    )

                    S = scalars_at(D, gall, vall, t)
                    out = emit_chain(m4c, WPh, HPh, SPh4, APm, S, clsV, mode="legacy")
                    nc.sync.dma_start(
                        out=cost_d[b, g0 : g0 + 128, :], in_=out[:]
                    )
                m4_batches.append((map4, m4c))

            # ---- packed remainder unit: rows 896:900 of all 4 batches ------
            # partitions 4b..4b+4 belong to batch b
            m4R = constp.tile([128, 4 * Q], f32, tag="m4R")
            for b in range(B_PER):
                src = pb_d[b][:].flatten()
                bcast4 = bass.AP(
                    tensor=src.tensor, offset=src.offset, ap=[[0, 4]] + list(src.ap)
                )
                nc.sync.dma_start(out=m4R[4 * b : 4 * b + 4, :], in_=bcast4)
            m4Rr = m4R[:].rearrange("p (q c) -> p c q", c=4)
            m4Rc = [m4Rr[:, c, :] for c in range(4)]
            WPhR, HPhR, SPh4R, APmR = derive_pred_maps(m4Rc, "R", constp)

            gtR = constp.tile([128, 1, 4], f32, tag="gtR")
            nc.gpsimd.memset(gtR[:], 0.5)
            vR = constp.tile([128, 1], f32, tag="vR")
            nc.gpsimd.memset(vR[:], 0.0)
            for b in range(B_PER):
                nc.sync.dma_start(
                    out=gtR[4 * b : 4 * b + 4, 0, :], in_=gb_d[b, 896:900, :]
                )
                nc.sync.dma_start(
                    out=vR[4 * b : 4 * b + 4, :],
                    in_=val_d[b, 896:900].rearrange("(p one) -> p one", one=1),
                )
            DR = derive_gt_scalars(gtR, vR, 1, "R", constp)

            clsTR = constp.tile([128, Q], f32, tag="clsTR")
            for b in range(B_PER):
                for k in range(PT):
                    p0 = k * 128
                    pw = 128 if k < 7 else 4
                    nc.sync.dma_start(
                        out=clsTR[4 * b : 4 * b + 4, p0 : p0 + pw],
                        in_=cls_d[b, p0 : p0 + pw, 896:900].rearrange("a b -> b a"),
                    )
            clsVR = chp.tile([128, Q], f32, tag="clsV")
            nc.scalar.activation(
                clsVR[:], clsTR[:], Act.Identity,
                bias=DR["twoV"][:, 0:1], scale=DR["negV"][:, 0:1],
            )
            SR = scalars_at(DR, gtR, vR, 0)
            outR = emit_chain(m4Rc, WPhR, HPhR, SPh4R, APmR, SR, clsVR, mode="legacy")
            for b in range(B_PER):
                nc.sync.dma_start(
                    out=cost_d[b, 896:900, :], in_=outR[4 * b : 4 * b + 4, :]
                )
    mybir.codegen_inst_isa_subclasses(nc)  # fill ISA bytes for custom-DVE ops
    _split_multi_waits(nc)
    return nc


def _get_nc():
    if "nc" not in _cached:
        _cached["nc"] = _build_nc()
    return _cached["nc"]


def _in_maps(pred_boxes, pred_cls, gt_boxes, gt_validity):
    maps = []
    for c in range(N_CORES):
        sl = slice(c * B_PER, (c + 1) * B_PER)
        maps.append(
            {
                "pred_boxes": np.ascontiguousarray(pred_boxes[sl], dtype=np.float32),
                "gt_boxes": np.ascontiguousarray(gt_boxes[sl], dtype=np.float32),
                "pred_cls": np.ascontiguousarray(pred_cls[sl], dtype=np.float32),
                "validity": np.ascontiguousarray(
                    gt_validity[sl].astype(np.float32)
                ),
            }
        )
    return maps


def kernel(pred_boxes, pred_cls, gt_boxes, gt_validity, _trace=False):
    from concourse import bass_utils

    nc = _get_nc()
    maps = _in_maps(pred_boxes, pred_cls, gt_boxes, gt_validity)
    res = bass_utils.run_bass_kernel_spmd(
        nc, maps, core_ids=list(range(N_CORES)), trace=_trace
    )
    out = np.concatenate([res.results[c]["cost"] for c in range(N_CORES)], axis=0)
    if _trace:
        _cached["last_result"] = res
    return out


# revision 30
# speedup vs baseline: 1.0520x; 1.0520x over previous
"""DETR-style matcher cost matrix on 8 Trainium2 NeuronCores.

cost[b, g, p] = -pred_cls[b, p, g]
                + mean(|pred_box[p] - gt_box[g]|)          (L1, 4 coords)
                + 1 - IoU + (area_c - union)/(area_c+eps)  (GIoU loss)
masked to zero where gt_validity[b, g] == 0.

Sharding: data-parallel over batch, 4 batches per core (B=32, 8 cores).

Layout per (batch, gt-tile of 128): [128 part = gt rows, 900 free = preds].
Per-pred values enter as partition-broadcast maps (fp16 for 2x DVE modes),
per-gt values as [128,1] fp32 scalars.  Identities used:
  wi0   = min(Px2,Gx2) - max(Px1,Gx1)        pre-relu intersection width
  wc    = (wp + wg) - wi0                    enclosing-box width
  l1*4  = (wc + hc) - (wi0 + hi0) = (wp+wg+hp+hg) - 2*(wi0+hi0)
  inter = relu(wi0)*relu(hi0)
  union = area_p + area_g - inter
  t2    = (area_c - union)/(area_c) ~= 1 - union/area_c   (eps folded)
  cost  = V*(0.25*SWH - 0.5*s2 + 2 - iou - union/area_c) - V*clsT
The fp32 division tail uses RECIPROCAL_APPROX_FAST (~51 ULP).
pred_cls.T comes via PE transposes into PSUM; ScalarE folds it to
V*(2 - clsT) in SBUF so the final combine is one scalar_tensor_tensor.
"""

import numpy as np

B, Q = 32, 900
N_CORES = 8
B_PER = B // N_CORES
EPS = 1e-7
GT = 8  # gt tiles per batch: 7 full x128 + 1 of 4 rows
PT = 8  # pred chunks of 128 (last = 4)

USE_CUSTOM = True  # authored fused DVE ops (W0_IOU_ANT / RELUMUL_ANT)
USE_GP = False  # offload some fp32 tail ops to GpSimd

_cached = {}


def _split_multi_waits(nc):
    """This neuronxcc build rejects >1 sync-wait per instruction. Split any
    instruction carrying N>1 waits by inserting N-1 wait-carrier nops before
    it on the same (in-order) engine stream."""
    import concourse.mybir as mybir

    for fn in nc.m.functions:
        for bb in fn.blocks:
            out = []
            for ins in bb.instructions:
                si = getattr(ins, "sync_info", None)
                waits = list(si.on_wait) if (si and si.on_wait) else []
                if len(waits) > 1:
                    si.on_wait = [waits[-1]]
                    for j, w in enumerate(waits[:-1]):
                        nop = mybir.InstNoOp(name=f"{ins.name}-sw{j}", ins=[], outs=[])
                        nop.engine = ins.engine
                        nop.sync_info = mybir.SyncInfo(on_wait=[w], on_update=[])
                        out.append(nop)
                out.append(ins)
            bb.instructions[:] = out


def _ensure_custom_ops():
    """Author two fused DVE ops and register them in dve_ops' tables:
      W0_IOU_ANT:  out = min(in0, s0) - max(in1, s1)
      RELUMUL_ANT: out = relu(in0) * relu(in1)
    """
    from concourse import dve_ops
    from concourse.dve_spec import Spec, Src0, Src1, C0, C1, minn, maxx, relu
    from concourse.dve_spec import lower, _has_src1
    from concourse.dve_uop import DveOpSpec

    if "W0_IOU_ANT" in dve_ops._SUB_OPCODE_FOR_NAME:
        return

    from concourse.dve_spec import C2

    def author(name, body, ref):
        spec = Spec(body=body, reference=ref)
        row = max(dve_ops._SUB_OPCODE_FOR_NAME.values()) + 1
        shas = {}
        for ver in ("v3", "v4"):
            uops = lower(spec, ver=ver)
            s = DveOpSpec(name=name, opcode=row, uops=uops, rd1_en=_has_src1(spec))
            shas[ver] = s.sha(ver)
        op = dve_ops.DveOp(name, spec, False, shas)
        dve_ops.OPS.append(op)
        dve_ops.CUSTOM_DVE_SPECS[name] = spec
        dve_ops._SUB_OPCODE_FOR_NAME[name] = row
        return op

    w0 = author(
        "W0_IOU_ANT",
        (minn(Src0, C0) - maxx(Src1, C1)) * C2,
        lambda in0, in1, s0, s1, imm2: (np.minimum(in0, s0) - np.maximum(in1, s1))
        * imm2,
    )
    rm = author(
        "RELUMUL_ANT",
        relu(Src0) * relu(Src1) * C2,
        lambda in0, in1, s0, s1, imm2: np.maximum(in0, 0.0)
        * np.maximum(in1, 0.0)
        * imm2,
    )
    return w0, rm


def _by_name(dve_ops, name):
    for op in dve_ops.OPS:
        if op.name == name:
            return op
    raise KeyError(name)


def _build_nc():
    import concourse.bass as bass
    from concourse import mybir, dve_ops
    from concourse.tile import TileContext
    from concourse.masks import make_identity

    if USE_CUSTOM:
        _ensure_custom_ops()
        W0 = _by_name(dve_ops, "W0_IOU_ANT")
        RM = _by_name(dve_ops, "RELUMUL_ANT")

    f32 = mybir.dt.float32
    f16 = mybir.dt.float16
    Alu = mybir.AluOpType
    Act = mybir.ActivationFunctionType

    nc = bass.Bass()
    pb_d = nc.dram_tensor("pred_boxes", [B_PER, Q, 4], f32, kind="ExternalInput")
    gb_d = nc.dram_tensor("gt_boxes", [B_PER, Q, 4], f32, kind="ExternalInput")
    cls_d = nc.dram_tensor("pred_cls", [B_PER, Q, Q], f32, kind="ExternalInput")
    val_d = nc.dram_tensor("validity", [B_PER, Q], f32, kind="ExternalInput")
    cost_d = nc.dram_tensor("cost", [B_PER, Q, Q], f32, kind="ExternalOutput")

    with TileContext(nc) as tc:
        with (
            tc.tile_pool(name="const", bufs=1) as constp,
            tc.tile_pool(name="batch", bufs=2) as batchp,
            tc.tile_pool(name="cls", bufs=3) as clsp,
            tc.tile_pool(name="chain", bufs=2) as chp,
            tc.tile_pool(name="outp", bufs=3) as outp,
            tc.tile_pool(name="psum", bufs=2, space="PSUM") as psp,
        ):
            ident = constp.tile([128, 128], f32)
            make_identity(nc, ident)
            onesrow = constp.tile([1, Q], f32)
            nc.gpsimd.memset(onesrow[:], 1.0)
            neg2 = constp.tile([1, 1], f32)
            nc.gpsimd.memset(neg2[:], -2.0)

            gp = nc.gpsimd if USE_GP else nc.vector

            # widths are carried scaled by SC=256 in fp16 to stay clear of
            # fp16 subnormals; SC folds back out via imm scalars downstream.
            SC = 256.0 if USE_CUSTOM else 1.0
            ISC2 = 1.0 / (SC * SC)
            hdt = f16 if USE_CUSTOM else f32

            def emit_chain(m4c, WPhX, HPhX, SPh4X, APmX, S, clsV, mode="psacc"):
                """One [128 gt x 900 pred] unit chain; returns the out tile."""
                stt = nc.vector.scalar_tensor_tensor
                wi0 = chp.tile([128, Q], hdt, tag="wi0")
                hi0 = chp.tile([128, Q], hdt, tag="hi0")
                if USE_CUSTOM:
                    nc.vector._custom_dve(
                        W0, out=wi0[:], in0=m4c[2], in1=m4c[0],
                        s0=S["Gx2"], s1=S["Gx1"], imm2=SC,
                    )
                    nc.vector._custom_dve(
                        W0, out=hi0[:], in0=m4c[3], in1=m4c[1],
                        s0=S["Gy2"], s1=S["Gy1"], imm2=SC,
                    )
                else:
                    Mx1 = chp.tile([128, Q], f32, tag="Mx1")
                    nc.vector.tensor_scalar_max(Mx1[:], m4c[0], S["Gx1"])
                    mx2 = chp.tile([128, Q], f32, tag="mx2")
                    nc.vector.tensor_scalar_min(mx2[:], m4c[2], S["Gx2"])
                    nc.vector.tensor_sub(wi0[:], mx2[:], Mx1[:])
                    My1 = chp.tile([128, Q], f32, tag="My1")
                    nc.vector.tensor_scalar_max(My1[:], m4c[1], S["Gy1"])
                    my2 = chp.tile([128, Q], f32, tag="my2")
                    nc.vector.tensor_scalar_min(my2[:], m4c[3], S["Gy2"])
                    nc.vector.tensor_sub(hi0[:], my2[:], My1[:])

                s2 = chp.tile([128, Q], hdt, tag="s2")
                nc.vector.tensor_add(s2[:], wi0[:], hi0[:])

                W = chp.tile([128, Q], hdt, tag="W")
                nc.scalar.activation(W[:], WPhX[:], Act.Identity, bias=S["WGs"])
                wc = chp.tile([128, Q], hdt, tag="wc")
                nc.vector.tensor_sub(wc[:], W[:], wi0[:])
                H = chp.tile([128, Q], hdt, tag="H")
                nc.scalar.activation(H[:], HPhX[:], Act.Identity, bias=S["HGs"])
                hc = chp.tile([128, Q], hdt, tag="hc")
                nc.vector.tensor_sub(hc[:], H[:], hi0[:])

                inter = chp.tile([128, Q], f32, tag="inter")
                areac = chp.tile([128, Q], f32, tag="areac")
                if USE_CUSTOM:
                    # whole division cluster SC^2-scaled; ratios cancel
                    nc.vector._custom_dve(
                        RM, out=inter[:], in0=wi0[:], in1=hi0[:], imm2=1.0
                    )
                    nc.vector.tensor_mul(areac[:], wc[:], hc[:])
                elif False:
                    wiR = chp.tile([128, Q], f32, tag="wiR")
                    nc.vector.tensor_scalar_max(wiR[:], wi0[:], 0.0)
                    hiR = chp.tile([128, Q], f32, tag="hiR")
                    nc.vector.tensor_scalar_max(hiR[:], hi0[:], 0.0)
                    nc.vector.tensor_mul(inter[:], wiR[:], hiR[:])
                    nc.vector.tensor_mul(areac[:], wc[:], hc[:])
                union = chp.tile([128, Q], f32, tag="union")
                stt(union[:], APmX[:], S["AGe"], inter[:], Alu.add, Alu.subtract)

                rcu = chp.tile([128, Q], f32, tag="rcu")
                nc.scalar.activation(rcu[:], union[:], Act.Ln)
                nc.scalar.activation(rcu[:], rcu[:], Act.Exp, scale=-1.0)
                rca = chp.tile([128, Q], f32, tag="rca")
                nc.scalar.activation(rca[:], areac[:], Act.Ln)
                nc.scalar.activation(rca[:], rca[:], Act.Exp, scale=-1.0)

                u1 = chp.tile([128, Q], f32, tag="u1")
                nc.vector.tensor_mul(u1[:], inter[:], rcu[:])
                t2m = chp.tile([128, Q], f32, tag="t2m")
                nc.vector.tensor_mul(t2m[:], union[:], rca[:])
                c1 = chp.tile([128, Q], f32, tag="c1")
                nc.vector.tensor_add(c1[:], u1[:], t2m[:])

                out = outp.tile([128, Q], f32, tag="out")
                if mode == "psacc":
                    # clsV = V*(SWH4 + 2 - clsT) from the PE-accumulated PSUM
                    q = chp.tile([128, Q], f32, tag="q")
                    stt(q[:], s2[:], 0.5 / SC, c1[:], Alu.mult, Alu.add)
                    stt(out[:], q[:], S["negV"], clsV[:], Alu.mult, Alu.add)
                else:
                    # clsV = V*(2 - clsT); l1 map terms still on the DVE side
                    SWH4 = constp.tile([128, Q], hdt, tag="SWH4")
                    nc.scalar.activation(
                        SWH4[:], SPh4X[:], Act.Identity, bias=S["SG4"]
                    )
                    c3 = constp.tile([128, Q], f32, tag="c3")
                    stt(c3[:], s2[:], -0.5 / SC, SWH4[:], Alu.mult, Alu.add)
                    c4 = constp.tile([128, Q], f32, tag="c4")
                    nc.vector.tensor_sub(c4[:], c3[:], c1[:])
                    stt(out[:], c4[:], S["V"], clsV[:], Alu.mult, Alu.add)
                return out

            def derive_pred_maps(m4c, tagsuf, pool):
                WPhX = pool.tile([128, Q], hdt, tag="WPh" + tagsuf)
                HPhX = pool.tile([128, Q], hdt, tag="HPh" + tagsuf)
                if USE_CUSTOM:
                    nc.vector._custom_dve(
                        W0, out=WPhX[:], in0=m4c[2], in1=m4c[0],
                        s0=1e30, s1=-1e30, imm2=SC,
                    )
                    nc.vector._custom_dve(
                        W0, out=HPhX[:], in0=m4c[3], in1=m4c[1],
                        s0=1e30, s1=-1e30, imm2=SC,
                    )
                else:
                    nc.vector.tensor_sub(WPhX[:], m4c[2], m4c[0])
                    nc.vector.tensor_sub(HPhX[:], m4c[3], m4c[1])
                SPsX = chp.tile([128, Q], hdt, tag="SPs")
                nc.vector.tensor_add(SPsX[:], WPhX[:], HPhX[:])
                SPh4X = pool.tile([128, Q], hdt, tag="SPh4" + tagsuf)
                nc.vector.tensor_scalar_mul(SPh4X[:], SPsX[:], 0.25 / SC)
                APmX = pool.tile([128, Q], f32, tag="APm" + tagsuf)
                if USE_CUSTOM:
                    nc.vector._custom_dve(
                        RM, out=APmX[:], in0=WPhX[:], in1=HPhX[:], imm2=1.0
                    )
                else:
                    nc.vector.tensor_mul(APmX[:], WPhX[:], HPhX[:])
                return WPhX, HPhX, SPh4X, APmX

            def derive_gt_scalars(gsrc, vsrc, n, tagsuf, pool):
                """gsrc [128,n,4] coords, vsrc [128,n] validity -> scalar tiles."""
                WGX = pool.tile([128, n], f32, tag="WG" + tagsuf)
                nc.vector.tensor_sub(WGX[:], gsrc[:, :, 2], gsrc[:, :, 0])
                HGX = pool.tile([128, n], f32, tag="HG" + tagsuf)
                nc.vector.tensor_sub(HGX[:], gsrc[:, :, 3], gsrc[:, :, 1])
                WGsX = pool.tile([128, n], f32, tag="WGs" + tagsuf)
                nc.vector.tensor_scalar_mul(WGsX[:], WGX[:], SC)
                HGsX = pool.tile([128, n], f32, tag="HGs" + tagsuf)
                nc.vector.tensor_scalar_mul(HGsX[:], HGX[:], SC)
                AGeX = pool.tile([128, n], f32, tag="AGe" + tagsuf)
                nc.vector.tensor_mul(AGeX[:], WGsX[:], HGsX[:])
                nc.vector.tensor_scalar_add(AGeX[:], AGeX[:], float(EPS) * SC * SC)
                SG4X = pool.tile([128, n], f32, tag="SG4" + tagsuf)
                nc.vector.tensor_add(SG4X[:], WGX[:], HGX[:])
                nc.vector.tensor_scalar_mul(SG4X[:], SG4X[:], 0.25)
                negVX = pool.tile([128, n], f32, tag="negV" + tagsuf)
                nc.vector.tensor_scalar_mul(negVX[:], vsrc[:], -1.0)
                twoVX = pool.tile([128, n], f32, tag="twoV" + tagsuf)
                nc.vector.tensor_scalar_mul(twoVX[:], vsrc[:], 2.0)
                return dict(WG=WGX, HG=HGX, AGe=AGeX, SG4=SG4X, WGs=WGsX,
                            HGs=HGsX, negV=negVX, twoV=twoVX)

            def scalars_at(D, gsrc, vsrc, t):
                return {
                    "Gx1": gsrc[:, t, 0:1], "Gy1": gsrc[:, t, 1:2],
                    "Gx2": gsrc[:, t, 2:3], "Gy2": gsrc[:, t, 3:4],
                    "WGs": D["WGs"][:, t : t + 1], "HGs": D["HGs"][:, t : t + 1],
                    "AGe": D["AGe"][:, t : t + 1], "SG4": D["SG4"][:, t : t + 1],
                    "V": vsrc[:, t : t + 1], "negV": D["negV"][:, t : t + 1],
                }

            m4_batches = []
            for b in range(B_PER):
                # ---- per-batch: pred maps (fp32 coords, partition-bcast) ----
                map4 = constp.tile([128, 4 * Q], f32, tag="map4")
                src = pb_d[b][:].flatten()  # [3600]
                bcast = bass.AP(
                    tensor=src.tensor, offset=src.offset, ap=[[0, 128]] + list(src.ap)
                )
                nc.sync.dma_start(out=map4[:], in_=bcast)
                m4 = map4[:].rearrange("p (q c) -> p c q", c=4)
                m4c = [m4[:, c, :] for c in range(4)]
                WPh, HPh, SPh4, APm = derive_pred_maps(m4c, "", batchp)

                # ---- per-batch: gt scalars ---------------------------------
                gall = batchp.tile([128, 7, 4], f32, tag="gall")
                nc.sync.dma_start(
                    out=gall[:],
                    in_=gb_d[b, 0:896, :].rearrange("(t p) c -> p t c", p=128),
                )
                vall = batchp.tile([128, 7], f32, tag="vall")
                nc.sync.dma_start(
                    out=vall[:],
                    in_=val_d[b, 0:896].rearrange("(t p) -> p t", p=128),
                )
                D = derive_gt_scalars(gall, vall, 7, "", batchp)

                # ---- 7 full gt-tile units ----------------------------------
                for t in range(7):
                    g0 = t * 128
                    clsin = clsp.tile([128, PT, 128], f32, tag="clsin")
                    for k in range(PT):
                        p0 = k * 128
                        pw = 128 if k < 7 else 4
                        nc.sync.dma_start(
                            out=clsin[0:pw, k, :],
                            in_=cls_d[b, p0 : p0 + pw, g0 : g0 + 128],
                        )
                    psA = psp.tile([128, 512], f32, tag="psA")
                    psB = psp.tile([128, 388], f32, tag="psB")
                    for k in range(PT):
                        p0 = k * 128
                        pw = 128 if k < 7 else 4
                        dst = (
                            psA[:, p0 : p0 + pw]
                            if p0 < 512
                            else psB[:, p0 - 512 : p0 - 512 + pw]
                        )
                        nc.tensor.transpose(dst, clsin[0:pw, k, :], ident[0:pw, 0:pw])

                    negVt = D["negV"][:, t : t + 1]
                    twoVt = D["twoV"][:, t : t + 1]
                    clsV = chp.tile([128, Q], f32, tag="clsV")
                    nc.scalar.activation(
                        clsV[:, 0:512], psA[:, :], Act.Identity, bias=twoVt, scale=negVt
                    )
                    nc.scalar.activation(
                        clsV[:, 512:900], psB[:, :], Act.Identity, bias=twoVt, scale=negVt
                    )

                    S = scalars_at(D, gall, vall, t)
                    out = emit_chain(m4c, WPh, HPh, SPh4, APm, S, clsV, mode="legacy")
                    nc.sync.dma_start(
                        out=cost_d[b, g0 : g0 + 128, :], in_=out[:]
                    )
                m4_batches.append((map4, m4c))

            # ---- packed remainder unit: rows 896:900 of all 4 batches ------
            # partitions 4b..4b+4 belong to batch b
            m4R = constp.tile([128, 4 * Q], f32, tag="m4R")
            for b in range(B_PER):
                src = pb_d[b][:].flatten()
                bcast4 = bass.AP(
                    tensor=src.tensor, offset=src.offset, ap=[[0, 4]] + list(src.ap)
                )
                nc.sync.dma_start(out=m4R[4 * b : 4 * b + 4, :], in_=bcast4)
            m4Rr = m4R[:].rearrange("p (q c) -> p c q", c=4)
            m4Rc = [m4Rr[:, c, :] for c in range(4)]
            WPhR, HPhR, SPh4R, APmR = derive_pred_maps(m4Rc, "R", constp)

            gtR = constp.tile([128, 1, 4], f32, tag="gtR")
            nc.gpsimd.memset(gtR[:], 0.5)
            vR = constp.tile([128, 1], f32, tag="vR")
            nc.gpsimd.memset(vR[:], 0.0)
            for b in range(B_PER):
                nc.sync.dma_start(
                    out=gtR[4 * b : 4 * b + 4, 0, :], in_=gb_d[b, 896:900, :]
                )
                nc.sync.dma_start(
                    out=vR[4 * b : 4 * b + 4, :],
                    in_=val_d[b, 896:900].rearrange("(p one) -> p one", one=1),
                )
            DR = derive_gt_scalars(gtR, vR, 1, "R", constp)

            clsTR = constp.tile([128, Q], f32, tag="clsTR")
            for b in range(B_PER):
                for k in range(PT):
                    p0 = k * 128
                    pw = 128 if k < 7 else 4
                    nc.sync.dma_start(
                        out=clsTR[4 * b : 4 * b + 4, p0 : p0 + pw],
                        in_=cls_d[b, p0 : p0 + pw, 896:900].rearrange("a b -> b a"),
                    )
            clsVR = chp.tile([128, Q], f32, tag="clsV")
            nc.scalar.activation(
                clsVR[:], clsTR[:], Act.Identity,
                bias=DR["twoV"][:, 0:1], scale=DR["negV"][:, 0:1],
            )
            SR = scalars_at(DR, gtR, vR, 0)
            outR = emit_chain(m4Rc, WPhR, HPhR, SPh4R, APmR, SR, clsVR, mode="legacy")
            for b in range(B_PER):
                nc.sync.dma_start(
                    out=cost_d[b, 896:900, :], in_=outR[4 * b : 4 * b + 4, :]
                )
    mybir.codegen_inst_isa_subclasses(nc)  # fill ISA bytes for custom-DVE ops
    _split_multi_waits(nc)
    return nc


def _get_nc():
    if "nc" not in _cached:
        _cached["nc"] = _build_nc()
    return _cached["nc"]


def _in_maps(pred_boxes, pred_cls, gt_boxes, gt_validity):
    maps = []
    for c in range(N_CORES):
        sl = slice(c * B_PER, (c + 1) * B_PER)
        maps.append(
            {
                "pred_boxes": np.ascontiguousarray(pred_boxes[sl], dtype=np.float32),
                "gt_boxes": np.ascontiguousarray(gt_boxes[sl], dtype=np.float32),
                "pred_cls": np.ascontiguousarray(pred_cls[sl], dtype=np.float32),
                "validity": np.ascontiguousarray(
                    gt_validity[sl].astype(np.float32)
                ),
            }
        )
    return maps


def kernel(pred_boxes, pred_cls, gt_boxes, gt_validity, _trace=False):
    from concourse import bass_utils

    nc = _get_nc()
    maps = _in_maps(pred_boxes, pred_cls, gt_boxes, gt_validity)
    res = bass_utils.run_bass_kernel_spmd(
        nc, maps, core_ids=list(range(N_CORES)), trace=_trace
    )
    out = np.concatenate([res.results[c]["cost"] for c in range(N_CORES)], axis=0)
    if _trace:
        _cached["last_result"] = res
    return out


# revision 31
# speedup vs baseline: 1.0529x; 1.0008x over previous
"""DETR-style matcher cost matrix on 8 Trainium2 NeuronCores.

cost[b, g, p] = -pred_cls[b, p, g]
                + mean(|pred_box[p] - gt_box[g]|)          (L1, 4 coords)
                + 1 - IoU + (area_c - union)/(area_c+eps)  (GIoU loss)
masked to zero where gt_validity[b, g] == 0.

Sharding: data-parallel over batch, 4 batches per core (B=32, 8 cores).

Layout per (batch, gt-tile of 128): [128 part = gt rows, 900 free = preds].
Per-pred values enter as partition-broadcast maps (fp16 for 2x DVE modes),
per-gt values as [128,1] fp32 scalars.  Identities used:
  wi0   = min(Px2,Gx2) - max(Px1,Gx1)        pre-relu intersection width
  wc    = (wp + wg) - wi0                    enclosing-box width
  l1*4  = (wc + hc) - (wi0 + hi0) = (wp+wg+hp+hg) - 2*(wi0+hi0)
  inter = relu(wi0)*relu(hi0)
  union = area_p + area_g - inter
  t2    = (area_c - union)/(area_c) ~= 1 - union/area_c   (eps folded)
  cost  = V*(0.25*SWH - 0.5*s2 + 2 - iou - union/area_c) - V*clsT
The fp32 division tail uses RECIPROCAL_APPROX_FAST (~51 ULP).
pred_cls.T comes via PE transposes into PSUM; ScalarE folds it to
V*(2 - clsT) in SBUF so the final combine is one scalar_tensor_tensor.
"""

import numpy as np

B, Q = 32, 900
N_CORES = 8
B_PER = B // N_CORES
EPS = 1e-7
GT = 8  # gt tiles per batch: 7 full x128 + 1 of 4 rows
PT = 8  # pred chunks of 128 (last = 4)

USE_CUSTOM = True  # authored fused DVE ops (W0_IOU_ANT / RELUMUL_ANT)
_cached = {}


def _split_multi_waits(nc):
    """This neuronxcc build rejects >1 sync-wait per instruction. Split any
    instruction carrying N>1 waits by inserting N-1 wait-carrier nops before
    it on the same (in-order) engine stream."""
    import concourse.mybir as mybir

    for fn in nc.m.functions:
        for bb in fn.blocks:
            out = []
            for ins in bb.instructions:
                si = getattr(ins, "sync_info", None)
                waits = list(si.on_wait) if (si and si.on_wait) else []
                if len(waits) > 1:
                    si.on_wait = [waits[-1]]
                    for j, w in enumerate(waits[:-1]):
                        nop = mybir.InstNoOp(name=f"{ins.name}-sw{j}", ins=[], outs=[])
                        nop.engine = ins.engine
                        nop.sync_info = mybir.SyncInfo(on_wait=[w], on_update=[])
                        out.append(nop)
                out.append(ins)
            bb.instructions[:] = out


def _ensure_custom_ops():
    """Author two fused DVE ops and register them in dve_ops' tables:
      W0_IOU_ANT:  out = min(in0, s0) - max(in1, s1)
      RELUMUL_ANT: out = relu(in0) * relu(in1)
    """
    from concourse import dve_ops
    from concourse.dve_spec import Spec, Src0, Src1, C0, C1, minn, maxx, relu
    from concourse.dve_spec import lower, _has_src1
    from concourse.dve_uop import DveOpSpec

    if "W0_IOU_ANT" in dve_ops._SUB_OPCODE_FOR_NAME:
        return

    from concourse.dve_spec import C2

    def author(name, body, ref):
        spec = Spec(body=body, reference=ref)
        row = max(dve_ops._SUB_OPCODE_FOR_NAME.values()) + 1
        shas = {}
        for ver in ("v3", "v4"):
            uops = lower(spec, ver=ver)
            s = DveOpSpec(name=name, opcode=row, uops=uops, rd1_en=_has_src1(spec))
            shas[ver] = s.sha(ver)
        op = dve_ops.DveOp(name, spec, False, shas)
        dve_ops.OPS.append(op)
        dve_ops.CUSTOM_DVE_SPECS[name] = spec
        dve_ops._SUB_OPCODE_FOR_NAME[name] = row
        return op

    w0 = author(
        "W0_IOU_ANT",
        (minn(Src0, C0) - maxx(Src1, C1)) * C2,
        lambda in0, in1, s0, s1, imm2: (np.minimum(in0, s0) - np.maximum(in1, s1))
        * imm2,
    )
    rm = author(
        "RELUMUL_ANT",
        relu(Src0) * relu(Src1) * C2,
        lambda in0, in1, s0, s1, imm2: np.maximum(in0, 0.0)
        * np.maximum(in1, 0.0)
        * imm2,
    )
    return w0, rm


def _by_name(dve_ops, name):
    for op in dve_ops.OPS:
        if op.name == name:
            return op
    raise KeyError(name)


def _build_nc():
    import concourse.bass as bass
    from concourse import mybir, dve_ops
    from concourse.tile import TileContext
    from concourse.masks import make_identity

    if USE_CUSTOM:
        _ensure_custom_ops()
        W0 = _by_name(dve_ops, "W0_IOU_ANT")
        RM = _by_name(dve_ops, "RELUMUL_ANT")

    f32 = mybir.dt.float32
    f16 = mybir.dt.float16
    Alu = mybir.AluOpType
    Act = mybir.ActivationFunctionType

    nc = bass.Bass()
    pb_d = nc.dram_tensor("pred_boxes", [B_PER, Q, 4], f32, kind="ExternalInput")
    gb_d = nc.dram_tensor("gt_boxes", [B_PER, Q, 4], f32, kind="ExternalInput")
    cls_d = nc.dram_tensor("pred_cls", [B_PER, Q, Q], f32, kind="ExternalInput")
    val_d = nc.dram_tensor("validity", [B_PER, Q], f32, kind="ExternalInput")
    cost_d = nc.dram_tensor("cost", [B_PER, Q, Q], f32, kind="ExternalOutput")

    with TileContext(nc) as tc:
        with (
            tc.tile_pool(name="const", bufs=1) as constp,
            tc.tile_pool(name="batch", bufs=2) as batchp,
            tc.tile_pool(name="cls", bufs=3) as clsp,
            tc.tile_pool(name="chain", bufs=2) as chp,
            tc.tile_pool(name="outp", bufs=3) as outp,
            tc.tile_pool(name="psum", bufs=2, space="PSUM") as psp,
        ):
            ident = constp.tile([128, 128], f32)
            make_identity(nc, ident)

            # widths are carried scaled by SC=256 in fp16 to stay clear of
            # fp16 subnormals; SC folds back out via imm scalars downstream.
            SC = 256.0 if USE_CUSTOM else 1.0
            ISC2 = 1.0 / (SC * SC)
            hdt = f16 if USE_CUSTOM else f32

            def emit_chain(m4c, WPhX, HPhX, SPh4X, APmX, S, clsV, mode="psacc"):
                """One [128 gt x 900 pred] unit chain; returns the out tile."""
                stt = nc.vector.scalar_tensor_tensor
                wi0 = chp.tile([128, Q], hdt, tag="wi0")
                hi0 = chp.tile([128, Q], hdt, tag="hi0")
                if USE_CUSTOM:
                    nc.vector._custom_dve(
                        W0, out=wi0[:], in0=m4c[2], in1=m4c[0],
                        s0=S["Gx2"], s1=S["Gx1"], imm2=SC,
                    )
                    nc.vector._custom_dve(
                        W0, out=hi0[:], in0=m4c[3], in1=m4c[1],
                        s0=S["Gy2"], s1=S["Gy1"], imm2=SC,
                    )
                else:
                    Mx1 = chp.tile([128, Q], f32, tag="Mx1")
                    nc.vector.tensor_scalar_max(Mx1[:], m4c[0], S["Gx1"])
                    mx2 = chp.tile([128, Q], f32, tag="mx2")
                    nc.vector.tensor_scalar_min(mx2[:], m4c[2], S["Gx2"])
                    nc.vector.tensor_sub(wi0[:], mx2[:], Mx1[:])
                    My1 = chp.tile([128, Q], f32, tag="My1")
                    nc.vector.tensor_scalar_max(My1[:], m4c[1], S["Gy1"])
                    my2 = chp.tile([128, Q], f32, tag="my2")
                    nc.vector.tensor_scalar_min(my2[:], m4c[3], S["Gy2"])
                    nc.vector.tensor_sub(hi0[:], my2[:], My1[:])

                s2 = chp.tile([128, Q], hdt, tag="s2")
                nc.vector.tensor_add(s2[:], wi0[:], hi0[:])

                W = chp.tile([128, Q], hdt, tag="W")
                nc.scalar.activation(W[:], WPhX[:], Act.Identity, bias=S["WGs"])
                wc = chp.tile([128, Q], hdt, tag="wc")
                nc.vector.tensor_sub(wc[:], W[:], wi0[:])
                H = chp.tile([128, Q], hdt, tag="H")
                nc.scalar.activation(H[:], HPhX[:], Act.Identity, bias=S["HGs"])
                hc = chp.tile([128, Q], hdt, tag="hc")
                nc.vector.tensor_sub(hc[:], H[:], hi0[:])

                inter = chp.tile([128, Q], f32, tag="inter")
                areac = chp.tile([128, Q], f32, tag="areac")
                if USE_CUSTOM:
                    # whole division cluster SC^2-scaled; ratios cancel
                    nc.vector._custom_dve(
                        RM, out=inter[:], in0=wi0[:], in1=hi0[:], imm2=1.0
                    )
                    nc.vector.tensor_mul(areac[:], wc[:], hc[:])
                elif False:
                    wiR = chp.tile([128, Q], f32, tag="wiR")
                    nc.vector.tensor_scalar_max(wiR[:], wi0[:], 0.0)
                    hiR = chp.tile([128, Q], f32, tag="hiR")
                    nc.vector.tensor_scalar_max(hiR[:], hi0[:], 0.0)
                    nc.vector.tensor_mul(inter[:], wiR[:], hiR[:])
                    nc.vector.tensor_mul(areac[:], wc[:], hc[:])
                union = chp.tile([128, Q], f32, tag="union")
                stt(union[:], APmX[:], S["AGe"], inter[:], Alu.add, Alu.subtract)

                rcu = chp.tile([128, Q], f32, tag="rcu")
                nc.scalar.activation(rcu[:], union[:], Act.Ln)
                nc.scalar.activation(rcu[:], rcu[:], Act.Exp, scale=-1.0)
                rca = chp.tile([128, Q], f32, tag="rca")
                nc.scalar.activation(rca[:], areac[:], Act.Ln)
                nc.scalar.activation(rca[:], rca[:], Act.Exp, scale=-1.0)

                u1 = chp.tile([128, Q], f32, tag="u1")
                nc.vector.tensor_mul(u1[:], inter[:], rcu[:])
                t2m = chp.tile([128, Q], f32, tag="t2m")
                nc.vector.tensor_mul(t2m[:], union[:], rca[:])
                c1 = chp.tile([128, Q], f32, tag="c1")
                nc.vector.tensor_add(c1[:], u1[:], t2m[:])

                out = outp.tile([128, Q], f32, tag="out")
                if mode == "psacc":
                    # clsV = V*(SWH4 + 2 - clsT) from the PE-accumulated PSUM
                    q = chp.tile([128, Q], f32, tag="q")
                    stt(q[:], s2[:], 0.5 / SC, c1[:], Alu.mult, Alu.add)
                    stt(out[:], q[:], S["negV"], clsV[:], Alu.mult, Alu.add)
                else:
                    # clsV = V*(2 - clsT); l1 map terms still on the DVE side
                    SWH4 = constp.tile([128, Q], hdt, tag="SWH4")
                    nc.scalar.activation(
                        SWH4[:], SPh4X[:], Act.Identity, bias=S["SG4"]
                    )
                    c3 = constp.tile([128, Q], f32, tag="c3")
                    stt(c3[:], s2[:], -0.5 / SC, SWH4[:], Alu.mult, Alu.add)
                    c4 = constp.tile([128, Q], f32, tag="c4")
                    nc.vector.tensor_sub(c4[:], c3[:], c1[:])
                    stt(out[:], c4[:], S["V"], clsV[:], Alu.mult, Alu.add)
                return out

            def derive_pred_maps(m4c, tagsuf, pool):
                WPhX = pool.tile([128, Q], hdt, tag="WPh" + tagsuf)
                HPhX = pool.tile([128, Q], hdt, tag="HPh" + tagsuf)
                if USE_CUSTOM:
                    nc.vector._custom_dve(
                        W0, out=WPhX[:], in0=m4c[2], in1=m4c[0],
                        s0=1e30, s1=-1e30, imm2=SC,
                    )
                    nc.vector._custom_dve(
                        W0, out=HPhX[:], in0=m4c[3], in1=m4c[1],
                        s0=1e30, s1=-1e30, imm2=SC,
                    )
                else:
                    nc.vector.tensor_sub(WPhX[:], m4c[2], m4c[0])
                    nc.vector.tensor_sub(HPhX[:], m4c[3], m4c[1])
                SPsX = chp.tile([128, Q], hdt, tag="SPs")
                nc.vector.tensor_add(SPsX[:], WPhX[:], HPhX[:])
                SPh4X = pool.tile([128, Q], hdt, tag="SPh4" + tagsuf)
                nc.vector.tensor_scalar_mul(SPh4X[:], SPsX[:], 0.25 / SC)
                APmX = pool.tile([128, Q], f32, tag="APm" + tagsuf)
                if USE_CUSTOM:
                    nc.vector._custom_dve(
                        RM, out=APmX[:], in0=WPhX[:], in1=HPhX[:], imm2=1.0
                    )
                else:
                    nc.vector.tensor_mul(APmX[:], WPhX[:], HPhX[:])
                return WPhX, HPhX, SPh4X, APmX

            def derive_gt_scalars(gsrc, vsrc, n, tagsuf, pool):
                """gsrc [128,n,4] coords, vsrc [128,n] validity -> scalar tiles."""
                WGX = pool.tile([128, n], f32, tag="WG" + tagsuf)
                nc.vector.tensor_sub(WGX[:], gsrc[:, :, 2], gsrc[:, :, 0])
                HGX = pool.tile([128, n], f32, tag="HG" + tagsuf)
                nc.vector.tensor_sub(HGX[:], gsrc[:, :, 3], gsrc[:, :, 1])
                WGsX = pool.tile([128, n], f32, tag="WGs" + tagsuf)
                nc.vector.tensor_scalar_mul(WGsX[:], WGX[:], SC)
                HGsX = pool.tile([128, n], f32, tag="HGs" + tagsuf)
                nc.vector.tensor_scalar_mul(HGsX[:], HGX[:], SC)
                AGeX = pool.tile([128, n], f32, tag="AGe" + tagsuf)
                nc.vector.tensor_mul(AGeX[:], WGsX[:], HGsX[:])
                nc.vector.tensor_scalar_add(AGeX[:], AGeX[:], float(EPS) * SC * SC)
                SG4X = pool.tile([128, n], f32, tag="SG4" + tagsuf)
                nc.vector.tensor_add(SG4X[:], WGX[:], HGX[:])
                nc.vector.tensor_scalar_mul(SG4X[:], SG4X[:], 0.25)
                negVX = pool.tile([128, n], f32, tag="negV" + tagsuf)
                nc.vector.tensor_scalar_mul(negVX[:], vsrc[:], -1.0)
                twoVX = pool.tile([128, n], f32, tag="twoV" + tagsuf)
                nc.vector.tensor_scalar_mul(twoVX[:], vsrc[:], 2.0)
                return dict(WG=WGX, HG=HGX, AGe=AGeX, SG4=SG4X, WGs=WGsX,
                            HGs=HGsX, negV=negVX, twoV=twoVX)

            def scalars_at(D, gsrc, vsrc, t):
                return {
                    "Gx1": gsrc[:, t, 0:1], "Gy1": gsrc[:, t, 1:2],
                    "Gx2": gsrc[:, t, 2:3], "Gy2": gsrc[:, t, 3:4],
                    "WGs": D["WGs"][:, t : t + 1], "HGs": D["HGs"][:, t : t + 1],
                    "AGe": D["AGe"][:, t : t + 1], "SG4": D["SG4"][:, t : t + 1],
                    "V": vsrc[:, t : t + 1], "negV": D["negV"][:, t : t + 1],
                }

            for b in range(B_PER):
                # ---- per-batch: pred maps (fp32 coords, partition-bcast) ----
                map4 = constp.tile([128, 4 * Q], f32, tag="map4")
                src = pb_d[b][:].flatten()  # [3600]
                bcast = bass.AP(
                    tensor=src.tensor, offset=src.offset, ap=[[0, 128]] + list(src.ap)
                )
                nc.sync.dma_start(out=map4[:], in_=bcast)
                m4 = map4[:].rearrange("p (q c) -> p c q", c=4)
                m4c = [m4[:, c, :] for c in range(4)]
                WPh, HPh, SPh4, APm = derive_pred_maps(m4c, "", batchp)

                # ---- per-batch: gt scalars ---------------------------------
                gall = batchp.tile([128, 7, 4], f32, tag="gall")
                nc.sync.dma_start(
                    out=gall[:],
                    in_=gb_d[b, 0:896, :].rearrange("(t p) c -> p t c", p=128),
                )
                vall = batchp.tile([128, 7], f32, tag="vall")
                nc.sync.dma_start(
                    out=vall[:],
                    in_=val_d[b, 0:896].rearrange("(t p) -> p t", p=128),
                )
                D = derive_gt_scalars(gall, vall, 7, "", batchp)

                # ---- 7 full gt-tile units ----------------------------------
                for t in range(7):
                    g0 = t * 128
                    clsin = clsp.tile([128, PT, 128], f32, tag="clsin")
                    for k in range(PT):
                        p0 = k * 128
                        pw = 128 if k < 7 else 4
                        nc.sync.dma_start(
                            out=clsin[0:pw, k, :],
                            in_=cls_d[b, p0 : p0 + pw, g0 : g0 + 128],
                        )
                    psA = psp.tile([128, 512], f32, tag="psA")
                    psB = psp.tile([128, 388], f32, tag="psB")
                    for k in range(PT):
                        p0 = k * 128
                        pw = 128 if k < 7 else 4
                        dst = (
                            psA[:, p0 : p0 + pw]
                            if p0 < 512
                            else psB[:, p0 - 512 : p0 - 512 + pw]
                        )
                        nc.tensor.transpose(dst, clsin[0:pw, k, :], ident[0:pw, 0:pw])

                    negVt = D["negV"][:, t : t + 1]
                    twoVt = D["twoV"][:, t : t + 1]
                    clsV = chp.tile([128, Q], f32, tag="clsV")
                    nc.scalar.activation(
                        clsV[:, 0:512], psA[:, :], Act.Identity, bias=twoVt, scale=negVt
                    )
                    nc.scalar.activation(
                        clsV[:, 512:900], psB[:, :], Act.Identity, bias=twoVt, scale=negVt
                    )

                    S = scalars_at(D, gall, vall, t)
                    out = emit_chain(m4c, WPh, HPh, SPh4, APm, S, clsV, mode="legacy")
                    nc.sync.dma_start(
                        out=cost_d[b, g0 : g0 + 128, :], in_=out[:]
                    )

            # ---- packed remainder unit: rows 896:900 of all 4 batches ------
            # partitions 4b..4b+4 belong to batch b
            m4R = constp.tile([128, 4 * Q], f32, tag="m4R")
            for b in range(B_PER):
                src = pb_d[b][:].flatten()
                bcast4 = bass.AP(
                    tensor=src.tensor, offset=src.offset, ap=[[0, 4]] + list(src.ap)
                )
                nc.sync.dma_start(out=m4R[4 * b : 4 * b + 4, :], in_=bcast4)
            m4Rr = m4R[:].rearrange("p (q c) -> p c q", c=4)
            m4Rc = [m4Rr[:, c, :] for c in range(4)]
            WPhR, HPhR, SPh4R, APmR = derive_pred_maps(m4Rc, "R", constp)

            gtR = constp.tile([128, 1, 4], f32, tag="gtR")
            nc.gpsimd.memset(gtR[:], 0.5)
            vR = constp.tile([128, 1], f32, tag="vR")
            nc.gpsimd.memset(vR[:], 0.0)
            for b in range(B_PER):
                nc.sync.dma_start(
                    out=gtR[4 * b : 4 * b + 4, 0, :], in_=gb_d[b, 896:900, :]
                )
                nc.sync.dma_start(
                    out=vR[4 * b : 4 * b + 4, :],
                    in_=val_d[b, 896:900].rearrange("(p one) -> p one", one=1),
                )
            DR = derive_gt_scalars(gtR, vR, 1, "R", constp)

            clsTR = constp.tile([128, Q], f32, tag="clsTR")
            for b in range(B_PER):
                for k in range(PT):
                    p0 = k * 128
                    pw = 128 if k < 7 else 4
                    nc.sync.dma_start(
                        out=clsTR[4 * b : 4 * b + 4, p0 : p0 + pw],
                        in_=cls_d[b, p0 : p0 + pw, 896:900].rearrange("a b -> b a"),
                    )
            clsVR = chp.tile([128, Q], f32, tag="clsV")
            nc.scalar.activation(
                clsVR[:], clsTR[:], Act.Identity,
                bias=DR["twoV"][:, 0:1], scale=DR["negV"][:, 0:1],
            )
            SR = scalars_at(DR, gtR, vR, 0)
            outR = emit_chain(m4Rc, WPhR, HPhR, SPh4R, APmR, SR, clsVR, mode="legacy")
            for b in range(B_PER):
                nc.sync.dma_start(
                    out=cost_d[b, 896:900, :], in_=outR[4 * b : 4 * b + 4, :]
                )
    mybir.codegen_inst_isa_subclasses(nc)  # fill ISA bytes for custom-DVE ops
    _split_multi_waits(nc)
    return nc


def _get_nc():
    if "nc" not in _cached:
        _cached["nc"] = _build_nc()
    return _cached["nc"]


def _in_maps(pred_boxes, pred_cls, gt_boxes, gt_validity):
    maps = []
    for c in range(N_CORES):
        sl = slice(c * B_PER, (c + 1) * B_PER)
        maps.append(
            {
                "pred_boxes": np.ascontiguousarray(pred_boxes[sl], dtype=np.float32),
                "gt_boxes": np.ascontiguousarray(gt_boxes[sl], dtype=np.float32),
                "pred_cls": np.ascontiguousarray(pred_cls[sl], dtype=np.float32),
                "validity": np.ascontiguousarray(
                    gt_validity[sl].astype(np.float32)
                ),
            }
        )
    return maps


def kernel(pred_boxes, pred_cls, gt_boxes, gt_validity, _trace=False):
    from concourse import bass_utils

    nc = _get_nc()
    maps = _in_maps(pred_boxes, pred_cls, gt_boxes, gt_validity)
    res = bass_utils.run_bass_kernel_spmd(
        nc, maps, core_ids=list(range(N_CORES)), trace=_trace
    )
    out = np.concatenate([res.results[c]["cost"] for c in range(N_CORES)], axis=0)
    if _trace:
        _cached["last_result"] = res
    return out
